# revision 1
# baseline (speedup 1.0000x reference)
"""LinearAttention Trainium2 Bass kernel.

kernel(**inputs) takes the full unsharded inputs from setup_inputs() and
returns the full output. Shards data-parallel over batch (b=8) across 8
NeuronCores; each core computes one batch item:

  qkv = w_qkv @ x            (layout B on chip: [n, 768], n on partitions)
  q = softmax_d(q); k = softmax_n(k)
  ctx[h] = ek_h^T @ v_h      (accumulated over n in PSUM; an appended ones
                              column yields sum_n ek for the k softmax)
  M^T = blockdiag(ctx/s_k)^T @ w_out^T   (folded once between passes)
  out = M @ eqnA + b_out     (eqnA = PE-transposed normalized exp(q))

Matmul operands are bf16; accumulation stays fp32 in PSUM.
"""

import numpy as np
import ml_dtypes

import concourse.bass as bass
import concourse.tile as tile
from concourse import bacc, mybir
from concourse.bass_utils import run_bass_kernel_spmd
from concourse.masks import make_identity

F32 = mybir.dt.float32
BF16 = mybir.dt.bfloat16
AF = mybir.ActivationFunctionType

C = 128
N = 16384
HEADS = 4
DH = 64
INNER = HEADS * DH          # 256
QKV = 3 * INNER             # 768
NB = 512
SUB = NB // 128
NBLK = N // NB              # 32
NSUB = N // 128             # 128


def build_nc():
    nc = bacc.Bacc("TRN2", target_bir_lowering=False, debug=False, num_devices=8)

    x = nc.dram_tensor("x", [C, N], BF16, kind="ExternalInput")
    wqT = nc.dram_tensor("wqT", [C, QKV], BF16, kind="ExternalInput")
    woT = nc.dram_tensor("woT", [INNER, C], BF16, kind="ExternalInput")
    bo = nc.dram_tensor("bo", [C, 1], F32, kind="ExternalInput")
    out = nc.dram_tensor("out", [C, N], F32, kind="ExternalOutput")

    with tile.TileContext(nc) as tc:
        with (
            tc.tile_pool(name="consts", bufs=1) as consts,
            tc.tile_pool(name="eqa", bufs=1) as eqa,
            tc.tile_pool(name="xin", bufs=4) as xin,
            tc.tile_pool(name="work", bufs=4) as work,
            tc.tile_pool(name="small", bufs=4) as small,
        ):
            wq_s = consts.tile([C, QKV], BF16)
            nc.sync.dma_start(out=wq_s, in_=wqT[:, :])
            wo_s = consts.tile([C, 2, C], BF16)
            nc.sync.dma_start(out=wo_s[:, 0, :], in_=woT[0:128, :])
            nc.sync.dma_start(out=wo_s[:, 1, :], in_=woT[128:256, :])
            bo_s = consts.tile([C, 1], F32)
            nc.sync.dma_start(out=bo_s, in_=bo[:, :])
            ident = consts.tile([C, C], BF16)
            make_identity(nc, ident)

            # layout-A normalized exp(q): [:, 0, :] = heads 0/1, [:, 1, :] = 2/3
            eqnA = eqa.tile([C, 2, N], BF16)
            MT01 = consts.tile([C, C], BF16)
            MT23 = consts.tile([C, C], BF16)

            with (
                tc.tile_pool(name="qkvp", bufs=2, space="PSUM") as qkvp,
                tc.tile_pool(name="trp", bufs=2, space="PSUM") as trp,
                tc.tile_pool(name="ctxp", bufs=1, space="PSUM") as ctxp,
            ):
                ctx01 = ctxp.tile([C, INNER + 1], F32)
                ctx23 = ctxp.tile([C, INNER + 1], F32)

                x_blk = None
                for t in range(NSUB):
                    blk, s = divmod(t, SUB)
                    if s == 0:
                        x_blk = xin.tile([C, NB], BF16, tag="x_blk")
                        nc.sync.dma_start(
                            out=x_blk, in_=x[:, blk * NB : (blk + 1) * NB]
                        )
                    xs = x_blk[:, s * 128 : (s + 1) * 128]

                    qkv = qkvp.tile([C, QKV], F32, tag="qkv")
                    nc.tensor.matmul(
                        qkv[:, 0:512], lhsT=xs, rhs=wq_s[:, 0:512],
                        start=True, stop=True, skip_group_check=True,
                    )
                    nc.tensor.matmul(
                        qkv[:, 512:768], lhsT=xs, rhs=wq_s[:, 512:768],
                        start=True, stop=True, skip_group_check=True,
                    )

                    # one exp over q|k halves; heads 0..3 = q, 4..7 = k
                    eqk = work.tile([C, 8, DH], BF16, tag="eqk")
                    nc.scalar.activation(eqk[:, :, :], qkv[:, 0:512], AF.Exp)

                    sq = small.tile([C, HEADS, 1], F32, tag="sq")
                    nc.vector.reduce_sum(
                        sq, eqk[:, 0:4, :], axis=mybir.AxisListType.X
                    )
                    rq = small.tile([C, HEADS, 1], F32, tag="rq")
                    nc.vector.reciprocal(rq, sq)
                    eqn = work.tile([C, HEADS, DH], BF16, tag="eqn")
                    nc.gpsimd.tensor_mul(
                        eqn, eqk[:, 0:4, :], rq.broadcast_to([C, HEADS, DH])
                    )

                    vt = work.tile([C, INNER + 1], BF16, tag="vt")
                    if t % 2 == 0:
                        nc.vector.tensor_copy(vt[:, 0:256], qkv[:, 512:768])
                    else:
                        nc.scalar.copy(vt[:, 0:256], qkv[:, 512:768])
                    nc.gpsimd.memset(vt[:, 256:257], 1.0)

                    nc.tensor.matmul(
                        ctx01, lhsT=eqk[:, 4:6, :], rhs=vt,
                        start=(t == 0), stop=(t == NSUB - 1), skip_group_check=True,
                    )
                    nc.tensor.matmul(
                        ctx23, lhsT=eqk[:, 6:8, :], rhs=vt,
                        start=(t == 0), stop=(t == NSUB - 1), skip_group_check=True,
                    )

                    tr = trp.tile([C, 2, C], BF16, tag="tr")
                    nc.tensor.transpose(tr[:, 0, :], eqn[:, 0:2, :], ident)
                    nc.tensor.transpose(tr[:, 1, :], eqn[:, 2:4, :], ident)
                    if t % 2 == 0:
                        nc.scalar.copy(eqnA[:, :, t * 128 : (t + 1) * 128], tr)
                    else:
                        nc.vector.tensor_copy(
                            eqnA[:, :, t * 128 : (t + 1) * 128], tr
                        )

                # ---- fold: MT = (blockdiag(ctx/s_k))^T @ w_out^T ----
                r01 = small.tile([C, 1], F32, tag="r01")
                r23 = small.tile([C, 1], F32, tag="r23")
                nc.vector.reciprocal(r01, ctx01[:, 256:257])
                nc.vector.reciprocal(r23, ctx23[:, 256:257])
                bd01 = consts.tile([C, C], BF16)
                bd23 = consts.tile([C, C], BF16)
                nc.vector.tensor_scalar_mul(
                    bd01[0:64, 0:64], ctx01[0:64, 0:64], r01[0:64, 0:1]
                )
                nc.vector.tensor_scalar_mul(
                    bd01[64:128, 64:128], ctx01[64:128, 64:128], r01[64:128, 0:1]
                )
                nc.vector.tensor_scalar_mul(bd01[0:64, 64:128], ctx01[0:64, 64:128], 0.0)
                nc.vector.tensor_scalar_mul(bd01[64:128, 0:64], ctx01[64:128, 0:64], 0.0)
                nc.vector.tensor_scalar_mul(
                    bd23[0:64, 0:64], ctx23[0:64, 128:192], r23[0:64, 0:1]
                )
                nc.vector.tensor_scalar_mul(
                    bd23[64:128, 64:128], ctx23[64:128, 192:256], r23[64:128, 0:1]
                )
                nc.vector.tensor_scalar_mul(bd23[0:64, 64:128], ctx23[0:64, 0:64], 0.0)
                nc.vector.tensor_scalar_mul(bd23[64:128, 0:64], ctx23[64:128, 0:64], 0.0)

                for pair, bd, mt in ((0, bd01, MT01), (1, bd23, MT23)):
                    tb = trp.tile([C, 2, C], BF16, tag="tr")
                    nc.tensor.transpose(tb[:, 0, :], bd, ident)
                    bdt = consts.tile([C, C], BF16, tag=f"bdt{pair}")
                    nc.vector.tensor_copy(bdt, tb[:, 0, :])
                    mtp = qkvp.tile([C, QKV], F32, tag="qkv")
                    nc.tensor.matmul(
                        mtp[:, 0:128], lhsT=bdt, rhs=wo_s[:, pair, :],
                        start=True, stop=True, skip_group_check=True,
                    )
                    nc.vector.tensor_copy(mt, mtp[:, 0:128])

            # ---- pass 2: out = MT^T @ eqnA + b ----
            with tc.tile_pool(name="finp", bufs=2, space="PSUM") as finp:
                for blk in range(NBLK):
                    nsl = slice(blk * NB, (blk + 1) * NB)
                    fin = finp.tile([C, NB], F32, tag="fin")
                    nc.tensor.matmul(
                        fin, lhsT=MT01, rhs=eqnA[:, 0, nsl],
                        start=True, stop=False, skip_group_check=True,
                    )
                    nc.tensor.matmul(
                        fin, lhsT=MT23, rhs=eqnA[:, 1, nsl],
                        start=False, stop=True, skip_group_check=True,
                    )
                    osb = work.tile([C, NB], F32, tag="osb")
                    if blk % 2 == 0:
                        nc.scalar.activation(
                            osb, fin, AF.Identity, bias=bo_s[:, 0:1], scale=1.0
                        )
                    else:
                        nc.vector.tensor_scalar_add(osb, fin, bo_s[:, 0:1])
                    nc.sync.dma_start(out=out[:, nsl], in_=osb)

    nc.compile()
    return nc


_NC_CACHE = None


def kernel(x, w_qkv, w_out, b_out):
    global _NC_CACHE
    if _NC_CACHE is None:
        _NC_CACHE = build_nc()
    nc = _NC_CACHE

    b = x.shape[0]
    bf = ml_dtypes.bfloat16
    wqT = np.ascontiguousarray(np.asarray(w_qkv, dtype=np.float32).T.astype(bf))
    woT = np.ascontiguousarray(np.asarray(w_out, dtype=np.float32).T.astype(bf))
    bo = np.ascontiguousarray(np.asarray(b_out, dtype=np.float32).reshape(C, 1))
    xb = np.asarray(x, dtype=np.float32).reshape(b, C, N).astype(bf)
    in_maps = [
        {"x": np.ascontiguousarray(xb[i]), "wqT": wqT, "woT": woT, "bo": bo}
        for i in range(b)
    ]
    res = run_bass_kernel_spmd(nc, in_maps, core_ids=list(range(b)))
    return np.stack(
        [res.results[i]["out"].reshape(C, 128, 128) for i in range(b)]
    ).astype(np.float32)



# revision 16
# speedup vs baseline: 1.2959x; 1.2959x over previous
"""LinearAttention Trainium2 Bass kernel (optimized v2).

kernel(**inputs) takes the full unsharded inputs from setup_inputs() and
returns the full output. Shards data-parallel over batch (b=8) across 8
NeuronCores; each core computes one batch item:

  qkv = w_qkv @ x            (layout A on chip: [n, 768], n on partitions)
  q = softmax_d(q); k = softmax_n(k)
  ctx[h] = ek_h^T @ [v_h | 1]  (per-head col-tiled matmuls, PSUM-accumulated
                                over n; the ones column yields s_k)
  MT = blockdiag(ctx/s_k) @ w_out^T  (+ bias folded into MT rows of head 0,
                                      exploiting sum_d softmax(q)=1)
  out = MT^T @ eqnA          (eqnA = PE-transposed normalized exp(q))

Pass 1 runs 64 iterations of 256 columns (2 subtiles fused) so every
non-PE op amortizes its fixed issue cost over 512-1024 elements.
Matmul operands are bf16; accumulation stays fp32 in PSUM. Output is
written bf16 and upcast on the host.
"""

import numpy as np
import ml_dtypes

import concourse.bass as bass
import concourse.tile as tile
from concourse import bacc, mybir
from concourse.bass_utils import run_bass_kernel_spmd
from concourse.masks import make_identity

F32 = mybir.dt.float32
BF16 = mybir.dt.bfloat16
AF = mybir.ActivationFunctionType

C = 128
N = 16384
HEADS = 4
DH = 64
INNER = HEADS * DH          # 256
QKV = 3 * INNER             # 768
IT_COLS = 256               # columns per pass-1 iteration (2 subtiles)
NIT = N // IT_COLS          # 64
XT_COLS = 1024              # columns per x DMA tile
OB = 512                    # pass-2 block columns
NOB = N // OB               # 32


def build_nc():
    nc = bacc.Bacc("TRN2", target_bir_lowering=False, debug=False, num_devices=8)

    x = nc.dram_tensor("x", [C, N], BF16, kind="ExternalInput")
    wqT = nc.dram_tensor("wqT", [C, QKV], BF16, kind="ExternalInput")
    woT = nc.dram_tensor("woT", [INNER, C], BF16, kind="ExternalInput")
    bo = nc.dram_tensor("bo", [1, C], BF16, kind="ExternalInput")
    out = nc.dram_tensor("out", [C, N], BF16, kind="ExternalOutput")

    with tile.TileContext(nc) as tc:
        with (
            tc.tile_pool(name="consts", bufs=1) as consts,
            tc.tile_pool(name="eqa", bufs=1) as eqa,
            tc.tile_pool(name="xin", bufs=3) as xin,
            tc.tile_pool(name="eqkp", bufs=3) as eqkp,
            tc.tile_pool(name="eqnp", bufs=3) as eqnp,
            tc.tile_pool(name="small", bufs=3) as small,
            tc.tile_pool(name="vtp", bufs=1) as vtp,
            tc.tile_pool(name="outp", bufs=6) as outp,
        ):
            wq_s = consts.tile([C, QKV], BF16)
            nc.sync.dma_start(out=wq_s, in_=wqT[:, :])
            wo_s = consts.tile([C, 2, C], BF16)
            nc.sync.dma_start(out=wo_s[:, 0, :], in_=woT[0:128, :])
            nc.sync.dma_start(out=wo_s[:, 1, :], in_=woT[128:256, :])
            bo_s = consts.tile([1, C], BF16)
            nc.sync.dma_start(out=bo_s, in_=bo[:, :])
            ones64 = consts.tile([1, 64], BF16)
            nc.gpsimd.memset(ones64, 1.0)
            ident = consts.tile([C, C], BF16)
            make_identity(nc, ident)

            # eqnA[p, pair, blk, 128]: transposed normalized exp(q).
            # partition p = d within pair (head-even d 0:64, head-odd 64:128)
            eqnA = eqa.tile([C, 2, N // 128, 128], BF16)
            MT01 = consts.tile([C, C], BF16)
            MT23 = consts.tile([C, C], BF16)

            # 3 v-tiles rotated manually so the ones column is set once.
            vts = []
            for i in range(3):
                vt = vtp.tile([C, 2, INNER + 1], BF16, tag=f"vt{i}")
                nc.gpsimd.memset(vt[:, :, INNER : INNER + 1], 1.0)
                vts.append(vt)

            with (
                tc.tile_pool(name="qkp", bufs=2, space="PSUM") as qkp,
                tc.tile_pool(name="vp", bufs=1, space="PSUM") as vp,
                tc.tile_pool(name="ctxp", bufs=1, space="PSUM") as ctxp,
                tc.tile_pool(name="trp", bufs=1, space="PSUM") as trp,
            ):
                # ctx_t[:, p, 0:257]: packed pair-context (d rows for both
                # heads of the pair; diag blocks used) + s_k in col 256.
                ctx_t = ctxp.tile([C, 2, 512], F32)

                x_t = None
                for it in range(NIT):
                    xo = it % (XT_COLS // IT_COLS)
                    if xo == 0:
                        x_t = xin.tile([C, XT_COLS], BF16, tag="x")
                        nc.sync.dma_start(
                            out=x_t, in_=x[:, it * IT_COLS : it * IT_COLS + XT_COLS]
                        )

                    # qk psum [128, 4, 256] fp32: q0 [0], k0 [1], q1 [2], k1 [3]
                    qk = qkp.tile([C, 4, 256], F32, tag="qk")
                    vps = vp.tile([C, 2, 256], F32, tag="v")
                    xs0 = x_t[:, xo * IT_COLS : xo * IT_COLS + 128]
                    xs1 = x_t[:, xo * IT_COLS + 128 : xo * IT_COLS + 256]
                    nc.tensor.matmul(
                        qk[:, 0:2, :], lhsT=xs0, rhs=wq_s[:, 0:512],
                        start=True, stop=True, skip_group_check=True,
                    )
                    nc.tensor.matmul(
                        qk[:, 2:4, :], lhsT=xs1, rhs=wq_s[:, 0:512],
                        start=True, stop=True, skip_group_check=True,
                    )
                    nc.tensor.matmul(
                        vps[:, 0, :], lhsT=xs0, rhs=wq_s[:, 512:768],
                        start=True, stop=True, skip_group_check=True,
                    )
                    nc.tensor.matmul(
                        vps[:, 1, :], lhsT=xs1, rhs=wq_s[:, 512:768],
                        start=True, stop=True, skip_group_check=True,
                    )

                    # eqk blocks: q0 [0], k0 [1], q1 [2], k1 [3]
                    eqk = eqkp.tile([C, 4, 256], BF16, tag="eqk")
                    nc.scalar.activation(eqk[:, :, :], qk[:, :, :], AF.Exp)

                    # v copy PSUM->SBUF; ones column preset outside loop
                    vt = vts[it % 3]
                    nc.vector.tensor_copy(vt[:, :, 0:INNER], vps[:, :, :])

                    # s_q per (subtile, head)
                    eqk_q = eqk[:, 0:4:2, :].rearrange(
                        "p s (h d) -> p s h d", h=HEADS
                    )
                    sq = small.tile([C, 2, HEADS, 1], F32, tag="sq")
                    rq = small.tile([C, 2, HEADS, 1], BF16, tag="rq")
                    nc.vector.reduce_sum(sq, eqk_q, axis=mybir.AxisListType.X)
                    with nc.allow_low_precision("elementwise recip, one rounding"):
                        nc.vector.reciprocal(rq, sq)

                    eqn = eqnp.tile([C, 2, HEADS, DH], BF16, tag="eqn")
                    nc.gpsimd.tensor_mul(
                        eqn, eqk_q, rq.broadcast_to([C, 2, HEADS, DH])
                    )

                    # ctx accumulation: packed pair matmuls (full array, F=257)
                    for s in range(2):
                        for p in range(2):
                            nc.tensor.matmul(
                                ctx_t[:, p, 0:257],
                                lhsT=eqk[:, 2 * s + 1, p * 128 : (p + 1) * 128],
                                rhs=vt[:, s, :],
                                start=(it == 0 and s == 0),
                                stop=(it == NIT - 1 and s == 1),
                                skip_group_check=True,
                            )

                    # transpose normalized q into layout B (bf16 psum)
                    tr = trp.tile([C, 2, 2, 128], BF16, tag="tr")
                    for p in range(2):
                        for s in range(2):
                            nc.tensor.transpose(
                                tr[:, p, s, :], eqn[:, s, 2 * p : 2 * p + 2, :], ident
                            )
                    nc.vector.tensor_copy(
                        eqnA[:, :, 2 * it : 2 * it + 2, :], tr[:, :, :, :]
                    )

                # ---- fold: MT = blockdiag(ctx/s_k) @ w_out^T (+bias) ----
                # (separate reciprocal per pair: strided multi-dim input APs
                # are mishandled by the Reciprocal lowering)
                r_k = small.tile([C, 2, 1], F32, tag="rk")
                nc.vector.reciprocal(r_k[:, 0, :], ctx_t[:, 0, 256:257])
                nc.vector.reciprocal(r_k[:, 1, :], ctx_t[:, 1, 256:257])
                bd = [
                    consts.tile([C, C], BF16, tag=f"bd{p}", name=f"bd{p}")
                    for p in range(2)
                ]
                for p in range(2):
                    nc.gpsimd.memset(bd[p], 0.0)
                for p in range(2):
                    # ctx_t columns are global inner indices: pair p's heads
                    # sit at e = 128p..128p+64 and 128p+64..128p+128.
                    nc.vector.tensor_scalar_mul(
                        bd[p][0:64, 0:64],
                        ctx_t[0:64, p, 128 * p : 128 * p + 64],
                        r_k[0:64, p, 0:1],
                    )
                    nc.vector.tensor_scalar_mul(
                        bd[p][64:128, 64:128],
                        ctx_t[64:128, p, 128 * p + 64 : 128 * p + 128],
                        r_k[64:128, p, 0:1],
                    )
                for p, mt in ((0, MT01), (1, MT23)):
                    tb = trp.tile([C, 2, 2, 128], BF16, tag="tr")
                    nc.tensor.transpose(tb[:, 0, 0, :], bd[p], ident)
                    bdt = consts.tile([C, C], BF16, tag=f"bdt{p}")
                    nc.vector.tensor_copy(bdt, tb[:, 0, 0, :])
                    mtp = qkp.tile([C, 4, 256], F32, tag="qk")
                    nc.tensor.matmul(
                        mtp[:, 0, 0:128], lhsT=bdt, rhs=wo_s[:, p, :],
                        start=True, stop=(p == 1), skip_group_check=True,
                    )
                    if p == 0:
                        # bias trick: sum_d eqn[d in head0] == 1, so adding
                        # b_out to MT01 rows 0:64 adds b_out to the output.
                        nc.tensor.matmul(
                            mtp[0:64, 0, 0:128], lhsT=ones64, rhs=bo_s,
                            start=False, stop=True, skip_group_check=True,
                        )
                    nc.vector.tensor_copy(mt, mtp[:, 0, 0:128])

            # ---- pass 2: out = MT^T @ eqnA ----
            with tc.tile_pool(name="finp", bufs=4, space="PSUM") as finp:
                for blk in range(NOB):
                    fin = finp.tile([C, OB], F32, tag="fin")
                    nc.tensor.matmul(
                        fin, lhsT=MT01, rhs=eqnA[:, 0, 4 * blk : 4 * blk + 4, :],
                        start=True, stop=False, skip_group_check=True,
                    )
                    nc.tensor.matmul(
                        fin, lhsT=MT23, rhs=eqnA[:, 1, 4 * blk : 4 * blk + 4, :],
                        start=False, stop=True, skip_group_check=True,
                    )
                    osb = outp.tile([C, OB], BF16, tag="osb")
                    nc.vector.tensor_copy(osb[:, 0:256], fin[:, 0:256])
                    nc.scalar.copy(osb[:, 256:512], fin[:, 256:512])
                    nc.sync.dma_start(
                        out=out[:, blk * OB : (blk + 1) * OB], in_=osb
                    )

    nc.compile()
    return nc


_NC_CACHE = None


def prep_in_maps(x, w_qkv, w_out, b_out):
    b = x.shape[0]
    bf = ml_dtypes.bfloat16
    wqT = np.ascontiguousarray(np.asarray(w_qkv, dtype=np.float32).T.astype(bf))
    woT = np.ascontiguousarray(np.asarray(w_out, dtype=np.float32).T.astype(bf))
    bo = np.ascontiguousarray(
        np.asarray(b_out, dtype=np.float32).reshape(1, C).astype(bf)
    )
    xb = np.asarray(x, dtype=np.float32).reshape(b, C, N).astype(bf)
    return [
        {"x": np.ascontiguousarray(xb[i]), "wqT": wqT, "woT": woT, "bo": bo}
        for i in range(b)
    ]


def kernel(x, w_qkv, w_out, b_out):
    global _NC_CACHE
    if _NC_CACHE is None:
        _NC_CACHE = build_nc()
    nc = _NC_CACHE

    b = x.shape[0]
    in_maps = prep_in_maps(x, w_qkv, w_out, b_out)
    res = run_bass_kernel_spmd(nc, in_maps, core_ids=list(range(b)))
    return np.stack(
        [
            res.results[i]["out"].astype(np.float32).reshape(C, 128, 128)
            for i in range(b)
        ]
    )


# revision 20
# speedup vs baseline: 1.3496x; 1.0414x over previous
"""LinearAttention Trainium2 Bass kernel (optimized v2).

kernel(**inputs) takes the full unsharded inputs from setup_inputs() and
returns the full output. Shards data-parallel over batch (b=8) across 8
NeuronCores; each core computes one batch item:

  qkv = w_qkv @ x            (layout A on chip: [n, 768], n on partitions)
  q = softmax_d(q); k = softmax_n(k)
  ctx[h] = ek_h^T @ [v_h | 1]  (per-head col-tiled matmuls, PSUM-accumulated
                                over n; the ones column yields s_k)
  MT = blockdiag(ctx/s_k) @ w_out^T  (+ bias folded into MT rows of head 0,
                                      exploiting sum_d softmax(q)=1)
  out = MT^T @ eqnA          (eqnA = PE-transposed normalized exp(q))

Pass 1 runs 64 iterations of 256 columns (2 subtiles fused) so every
non-PE op amortizes its fixed issue cost over 512-1024 elements.
Matmul operands are bf16; accumulation stays fp32 in PSUM. Output is
written bf16 and upcast on the host.
"""

import numpy as np
import ml_dtypes

import concourse.bass as bass
import concourse.tile as tile
from concourse import bacc, mybir
from concourse.bass_utils import run_bass_kernel_spmd
from concourse.masks import make_identity

F32 = mybir.dt.float32
BF16 = mybir.dt.bfloat16
AF = mybir.ActivationFunctionType

C = 128
N = 16384
HEADS = 4
DH = 64
INNER = HEADS * DH          # 256
QKV = 3 * INNER             # 768
IT_COLS = 256               # columns per pass-1 iteration (2 subtiles)
NIT = N // IT_COLS          # 64
XT_COLS = 1024              # columns per x DMA tile
OB = 512                    # pass-2 block columns
NOB = N // OB               # 32


def build_nc():
    nc = bacc.Bacc("TRN2", target_bir_lowering=False, debug=False, num_devices=8)

    x = nc.dram_tensor("x", [C, N], BF16, kind="ExternalInput")
    wqT = nc.dram_tensor("wqT", [C, QKV], BF16, kind="ExternalInput")
    woT = nc.dram_tensor("woT", [INNER, C], BF16, kind="ExternalInput")
    bo = nc.dram_tensor("bo", [1, C], BF16, kind="ExternalInput")
    out = nc.dram_tensor("out", [C, N], BF16, kind="ExternalOutput")

    with tile.TileContext(nc) as tc:
        with (
            tc.tile_pool(name="consts", bufs=1) as consts,
            tc.tile_pool(name="eqa", bufs=1) as eqa,
            tc.tile_pool(name="xin", bufs=3) as xin,
            tc.tile_pool(name="eqkp", bufs=3) as eqkp,
            tc.tile_pool(name="eqnp", bufs=3) as eqnp,
            tc.tile_pool(name="small", bufs=3) as small,
            tc.tile_pool(name="vtp", bufs=1) as vtp,
            tc.tile_pool(name="outp", bufs=6) as outp,
        ):
            wq_s = consts.tile([C, QKV], BF16)
            nc.sync.dma_start(out=wq_s, in_=wqT[:, :])
            wo_s = consts.tile([C, 2, C], BF16)
            nc.sync.dma_start(out=wo_s[:, 0, :], in_=woT[0:128, :])
            nc.sync.dma_start(out=wo_s[:, 1, :], in_=woT[128:256, :])
            bo_s = consts.tile([1, C], BF16)
            nc.sync.dma_start(out=bo_s, in_=bo[:, :])
            ones64 = consts.tile([1, 64], BF16)
            nc.gpsimd.memset(ones64, 1.0)
            ident = consts.tile([C, C], BF16)
            make_identity(nc, ident)

            # eqnA[p, pair, blk, 128]: transposed normalized exp(q).
            # partition p = d within pair (head-even d 0:64, head-odd 64:128)
            eqnA = eqa.tile([C, 2, N // 128, 128], BF16)
            MT01 = consts.tile([C, C], BF16)
            MT23 = consts.tile([C, C], BF16)

            # 3 v-tiles rotated manually so the ones column is set once.
            vts = []
            for i in range(3):
                vt = vtp.tile([C, 2, INNER + 1], BF16, tag=f"vt{i}")
                nc.gpsimd.memset(vt[:, :, INNER : INNER + 1], 1.0)
                vts.append(vt)
            bd = [
                consts.tile([C, C], BF16, tag=f"bd{p}", name=f"bd{p}")
                for p in range(2)
            ]
            for p in range(2):
                nc.gpsimd.memset(bd[p], 0.0)

            with (
                tc.tile_pool(name="qkp", bufs=2, space="PSUM") as qkp,
                tc.tile_pool(name="vp", bufs=1, space="PSUM") as vp,
                tc.tile_pool(name="ctxp", bufs=1, space="PSUM") as ctxp,
                tc.tile_pool(name="trp", bufs=1, space="PSUM") as trp,
            ):
                # ctx_t[:, p, 0:257]: packed pair-context (d rows for both
                # heads of the pair; diag blocks used) + s_k in col 256.
                ctx_t = ctxp.tile([C, 2, 512], F32)

                x_t = None
                for it in range(NIT):
                    xo = it % (XT_COLS // IT_COLS)
                    if xo == 0:
                        x_t = xin.tile([C, XT_COLS], BF16, tag="x")
                        nc.sync.dma_start(
                            out=x_t, in_=x[:, it * IT_COLS : it * IT_COLS + XT_COLS]
                        )

                    # qk psum [128, 4, 256] fp32: q0 [0], k0 [1], q1 [2], k1 [3]
                    qk = qkp.tile([C, 4, 256], F32, tag="qk")
                    vps = vp.tile([C, 2, 256], F32, tag="v")
                    xs0 = x_t[:, xo * IT_COLS : xo * IT_COLS + 128]
                    xs1 = x_t[:, xo * IT_COLS + 128 : xo * IT_COLS + 256]
                    nc.tensor.matmul(
                        qk[:, 0:2, :], lhsT=xs0, rhs=wq_s[:, 0:512],
                        start=True, stop=True, skip_group_check=True,
                    )
                    nc.tensor.matmul(
                        qk[:, 2:4, :], lhsT=xs1, rhs=wq_s[:, 0:512],
                        start=True, stop=True, skip_group_check=True,
                    )
                    nc.tensor.matmul(
                        vps[:, 0, :], lhsT=xs0, rhs=wq_s[:, 512:768],
                        start=True, stop=True, skip_group_check=True,
                    )
                    nc.tensor.matmul(
                        vps[:, 1, :], lhsT=xs1, rhs=wq_s[:, 512:768],
                        start=True, stop=True, skip_group_check=True,
                    )

                    # eqk blocks: q0 [0], k0 [1], q1 [2], k1 [3]
                    eqk = eqkp.tile([C, 4, 256], BF16, tag="eqk")
                    nc.scalar.activation(eqk[:, :, :], qk[:, :, :], AF.Exp)

                    # v copy PSUM->SBUF; ones column preset outside loop
                    vt = vts[it % 3]
                    nc.vector.tensor_copy(vt[:, :, 0:INNER], vps[:, :, :])

                    # s_q per (subtile, head); bf16 keeps DVE in 2x mode
                    # (HW reduce accumulates in fp32 internally)
                    eqk_q = eqk[:, 0:4:2, :].rearrange(
                        "p s (h d) -> p s h d", h=HEADS
                    )
                    sq = small.tile([C, 2, HEADS], BF16, tag="sq")
                    rq = small.tile([C, 2, HEADS], BF16, tag="rq")
                    with nc.allow_low_precision("softmax denom tolerates bf16"):
                        nc.vector.reduce_sum(sq, eqk_q, axis=mybir.AxisListType.X)
                        nc.vector.reciprocal(rq, sq)

                    eqn = eqnp.tile([C, 2, HEADS, DH], BF16, tag="eqn")
                    nc.gpsimd.tensor_mul(
                        eqn, eqk_q,
                        rq.unsqueeze(3).broadcast_to([C, 2, HEADS, DH]),
                    )

                    # ctx accumulation: packed pair matmuls (full array, F=257)
                    for s in range(2):
                        for p in range(2):
                            nc.tensor.matmul(
                                ctx_t[:, p, 0:257],
                                lhsT=eqk[:, 2 * s + 1, p * 128 : (p + 1) * 128],
                                rhs=vt[:, s, :],
                                start=(it == 0 and s == 0),
                                stop=(it == NIT - 1 and s == 1),
                                skip_group_check=True,
                            )

                    # transpose normalized q into layout B (bf16 psum)
                    tr = trp.tile([C, 2, 2, 128], BF16, tag="tr")
                    for p in range(2):
                        for s in range(2):
                            nc.tensor.transpose(
                                tr[:, p, s, :], eqn[:, s, 2 * p : 2 * p + 2, :], ident
                            )
                    nc.vector.tensor_copy(
                        eqnA[:, :, 2 * it : 2 * it + 2, :], tr[:, :, :, :]
                    )

                # PE keep-alive through the fold: chained accumulating
                # matmuls into the retired v bank so HAM stays at 2.4 GHz
                # for pass 2 (they are never read).
                dmy = vp.tile([C, 2, 256], F32, tag="v")
                for i in range(24):
                    nc.tensor.matmul(
                        dmy[:, 0:2, :], lhsT=ident, rhs=wq_s[:, 0:512],
                        start=(i == 0), stop=(i == 23), skip_group_check=True,
                    )

                # ---- fold: MT = blockdiag(ctx/s_k) @ w_out^T (+bias) ----
                # (separate reciprocal per pair: strided multi-dim input APs
                # are mishandled by the Reciprocal lowering)
                r_k = small.tile([C, 2, 1], F32, tag="rk")
                nc.vector.reciprocal(r_k[:, 0, :], ctx_t[:, 0, 256:257])
                nc.vector.reciprocal(r_k[:, 1, :], ctx_t[:, 1, 256:257])
                for p in range(2):
                    # ctx_t columns are global inner indices: pair p's heads
                    # sit at e = 128p..128p+64 and 128p+64..128p+128.
                    nc.vector.tensor_scalar_mul(
                        bd[p][0:64, 0:64],
                        ctx_t[0:64, p, 128 * p : 128 * p + 64],
                        r_k[0:64, p, 0:1],
                    )
                    nc.vector.tensor_scalar_mul(
                        bd[p][64:128, 64:128],
                        ctx_t[64:128, p, 128 * p + 64 : 128 * p + 128],
                        r_k[64:128, p, 0:1],
                    )
                for p, mt in ((0, MT01), (1, MT23)):
                    tb = trp.tile([C, 2, 2, 128], BF16, tag="tr")
                    nc.tensor.transpose(tb[:, 0, 0, :], bd[p], ident)
                    bdt = consts.tile([C, C], BF16, tag=f"bdt{p}")
                    nc.vector.tensor_copy(bdt, tb[:, 0, 0, :])
                    mtp = qkp.tile([C, 4, 256], F32, tag="qk")
                    nc.tensor.matmul(
                        mtp[:, 0, 0:128], lhsT=bdt, rhs=wo_s[:, p, :],
                        start=True, stop=(p == 1), skip_group_check=True,
                    )
                    if p == 0:
                        # bias trick: sum_d eqn[d in head0] == 1, so adding
                        # b_out to MT01 rows 0:64 adds b_out to the output.
                        nc.tensor.matmul(
                            mtp[0:64, 0, 0:128], lhsT=ones64, rhs=bo_s,
                            start=False, stop=True, skip_group_check=True,
                        )
                    nc.vector.tensor_copy(mt, mtp[:, 0, 0:128])

            # ---- pass 2: out = MT^T @ eqnA  (1024-col blocks) ----
            with tc.tile_pool(name="finp", bufs=3, space="PSUM") as finp:
                for blk in range(N // 1024):
                    fin = finp.tile([C, 2, OB], F32, tag="fin")
                    for hb in range(2):
                        nsl = slice(8 * blk + 4 * hb, 8 * blk + 4 * hb + 4)
                        nc.tensor.matmul(
                            fin[:, hb, :], lhsT=MT01, rhs=eqnA[:, 0, nsl, :],
                            start=True, stop=False, skip_group_check=True,
                        )
                        nc.tensor.matmul(
                            fin[:, hb, :], lhsT=MT23, rhs=eqnA[:, 1, nsl, :],
                            start=False, stop=True, skip_group_check=True,
                        )
                    osb = outp.tile([C, 2, OB], BF16, tag="osb")
                    nc.vector.tensor_copy(osb[:, 0, :], fin[:, 0, :])
                    nc.scalar.copy(osb[:, 1, :], fin[:, 1, :])
                    nc.sync.dma_start(
                        out=out[:, blk * 1024 : (blk + 1) * 1024], in_=osb
                    )

    nc.compile()
    return nc


_NC_CACHE = None


def prep_in_maps(x, w_qkv, w_out, b_out):
    b = x.shape[0]
    bf = ml_dtypes.bfloat16
    wqT = np.ascontiguousarray(np.asarray(w_qkv, dtype=np.float32).T.astype(bf))
    woT = np.ascontiguousarray(np.asarray(w_out, dtype=np.float32).T.astype(bf))
    bo = np.ascontiguousarray(
        np.asarray(b_out, dtype=np.float32).reshape(1, C).astype(bf)
    )
    xb = np.asarray(x, dtype=np.float32).reshape(b, C, N).astype(bf)
    return [
        {"x": np.ascontiguousarray(xb[i]), "wqT": wqT, "woT": woT, "bo": bo}
        for i in range(b)
    ]


def kernel(x, w_qkv, w_out, b_out):
    global _NC_CACHE
    if _NC_CACHE is None:
        _NC_CACHE = build_nc()
    nc = _NC_CACHE

    b = x.shape[0]
    in_maps = prep_in_maps(x, w_qkv, w_out, b_out)
    res = run_bass_kernel_spmd(nc, in_maps, core_ids=list(range(b)))
    return np.stack(
        [
            res.results[i]["out"].astype(np.float32).reshape(C, 128, 128)
            for i in range(b)
        ]
    )
